# revision 17
# baseline (speedup 1.0000x reference)
"""KGram MLP seq model (embedding_lookup) on 8 Trainium2 NeuronCores.

Computation: emb[s,b] = sum_j W1t[token(s,b,j) + j*V] + b1 ; h = SiLU(emb)
             logits = h @ W2 + b2                      -> (2048, 2, 32000) f32

Sharding: vocab-parallel. Each core holds W2 columns [c*4000, (c+1)*4000),
computes the full h (the 3-row gathers are cheap and hidden under the
matmul), and produces its (4096, 4000) logits slice; the host concatenates.

Device pipeline per core (all positions flattened to 4096 = 2048*2 rows):
  - W1t rows are deduplicated on host to a compact bf16 table (<= 3*4098 rows,
    each pre-biased with b1/3 so the 3-row sum lands on emb+b1 directly) and
    gathered on-device with dma_gather(transpose=True), which lands the
    embeddings in [embed%128 (partition), embed//128, position] layout --
    exactly what the matmul needs as the stationary operand.
  - 3-row sum in bf16 (DVE); sigmoid on ACT; h128 = (2^7 e)*sigmoid(e) in one
    DVE scalar_tensor_tensor, fp8e4 out (2^7 pre-scale keeps fp8 normal).
  - h128 @ (2^4 W2) in fp8e4 DoubleRow (2 rows/cell), fp32 PSUM accumulate
    over 4 k-pairs. For the ACT-evicted n-tiles, +b2 is folded in as a final
    rank-1 bf16 matmul (ones x 2^11 b2).
  - Eviction is split to balance engines: DVE n-tiles fuse the 2^-11 descale
    and +b2 (scalar_tensor_tensor); ACT n-tiles are a scaled Copy. bf16 out,
    DMA to DRAM; host concatenates and casts to f32.
"""

import numpy as np
import ml_dtypes

VOCAB = 32000
KGRAM = 3
EMBED = 1024
SEQ = 2048
BATCH = 2
POS = SEQ * BATCH            # 4096 flattened positions (s-major, batch minor)
NCORES = 8
VSH = VOCAB // NCORES        # 4000 vocab columns per core
UPAD = 4098                  # padded unique-token count (4096 tokens + pad 0 + slack)
RTAB = KGRAM * UPAD          # compact table rows (12294 < int16 max)
P = 128
CB = EMBED // P              # 8 embed column blocks
KT = EMBED // P              # 8 contraction tiles
CHUNK = 256                  # positions per gather chunk
NCH = POS // CHUNK           # 16
GIDX = KGRAM * CHUNK         # 768 gather indices per chunk
IDXC = GIDX // 16            # 48 idx columns per chunk
NT = [512] * 7 + [416]       # vocab n-tile sizes (sum 4000)
NOFF = [0, 512, 1024, 1536, 2048, 2560, 3072, 3584]
NDVE = 3                     # n-tiles evicted on DVE (fused +b2); rest on ACT

HSC = 128.0                  # h pre-scale (power of 2; keeps fp8e4 normal)
WSC = 16.0                   # W2 pre-scale
OSC = 1.0 / (HSC * WSC)      # eviction descale

_BF16 = ml_dtypes.bfloat16
_FP8 = ml_dtypes.float8_e4m3

_CACHED_NC = None


def _build_nc():
    """Build + compile the per-core Bass program (identical on all 8 cores)."""
    from contextlib import ExitStack

    import concourse.bacc as bacc
    import concourse.tile as tile
    import concourse.mybir as mybir

    dtbf = mybir.dt.bfloat16
    dtf = mybir.dt.float32
    dt8 = mybir.dt.float8e4
    dti = mybir.dt.int16

    nc = bacc.Bacc("TRN2", target_bir_lowering=False, debug=False,
                   num_devices=NCORES)

    table = nc.dram_tensor("table", [RTAB, EMBED], dtbf, kind="ExternalInput")
    idx = nc.dram_tensor("idx", [P, NCH * IDXC], dti, kind="ExternalInput")
    w2b = nc.dram_tensor("w2b", [P, KT * VSH], dt8, kind="ExternalInput")
    b2r = nc.dram_tensor("b2r", [P, NOFF[NDVE]], dtf, kind="ExternalInput")
    b2m = nc.dram_tensor("b2m", [1, VSH], dtbf, kind="ExternalInput")
    out = nc.dram_tensor("out", [POS, VSH], dtbf, kind="ExternalOutput")

    sigm = mybir.ActivationFunctionType.Sigmoid
    copyf = mybir.ActivationFunctionType.Copy
    mul_op = mybir.AluOpType.mult
    add_op = mybir.AluOpType.add
    dr = mybir.MatmulPerfMode.DoubleRow

    with tile.TileContext(nc) as tc, ExitStack() as ctx:
        const = ctx.enter_context(tc.tile_pool(name="const", bufs=1))
        # small loads first: the first gather needs idxs, and the 4.1MB w2s
        # transfer would otherwise delay it ~13us on the SP DMA ring
        idxs = const.tile([P, NCH * IDXC], dti, tag="idxs")
        nc.sync.dma_start(idxs[:], idx.ap())
        b2s = const.tile([P, NOFF[NDVE]], dtf, tag="b2s")
        nc.sync.dma_start(b2s[:], b2r.ap())
        b2ms = const.tile([1, VSH], dtbf, tag="b2ms")
        nc.sync.dma_start(b2ms[:], b2m.ap())
        w2s = const.tile([P, KT, VSH], dt8, tag="w2s")
        nc.sync.dma_start(w2s[:], w2b.ap())
        ones1 = const.tile([1, P], dtbf, tag="ones1")
        nc.vector.memset(ones1[:], 1.0)

        gpool = ctx.enter_context(tc.tile_pool(name="g", bufs=3))
        spool = ctx.enter_context(tc.tile_pool(name="s", bufs=3))
        hpool = ctx.enter_context(tc.tile_pool(name="h", bufs=3))
        opool = ctx.enter_context(tc.tile_pool(name="o", bufs=3))
        psum = ctx.enter_context(tc.tile_pool(name="ps", bufs=8, space="PSUM"))

        for c in range(NCH):
            g = gpool.tile([P, CB, GIDX], dtbf, tag="g")
            nc.gpsimd.dma_gather(
                g[:], table.ap(), idxs[:, c * IDXC:(c + 1) * IDXC],
                GIDX, GIDX, EMBED, transpose=True,
            )
            s1 = spool.tile([P, CB, CHUNK], dtbf, tag="s1")
            nc.vector.tensor_add(s1[:], g[:, :, 0:CHUNK], g[:, :, CHUNK:2 * CHUNK])
            s2 = spool.tile([P, CB, CHUNK], dtbf, tag="s2")
            nc.vector.tensor_add(s2[:], s1[:], g[:, :, 2 * CHUNK:3 * CHUNK])
            # e = s2 already includes b1 (folded into the table rows)
            sg = spool.tile([P, CB, CHUNK], dtbf, tag="sg")
            nc.scalar.activation(sg[:], s2[:], sigm)
            # h128 = (HSC * e) * sigmoid(e)  -> fp8e4
            h = hpool.tile([P, CB, CHUNK], dt8, tag="h")
            nc.vector.scalar_tensor_tensor(h[:], s2[:], HSC, sg[:],
                                           op0=mul_op, op1=mul_op)

            ob = opool.tile([P, CHUNK // P, VSH], dtbf, tag="o")
            for m in range(CHUNK // P):
                o = ob[:, m, :]
                pts = [psum.tile([P, 512], dtf, tag="ps", name=f"ps{n}")
                       for n in range(8)]
                for kk in range(KT // 2):
                    lhsT = h[:, 2 * kk:2 * kk + 2, m * P:(m + 1) * P]
                    for n in range(8):
                        mm = nc.tensor.matmul(
                            pts[n][:, :NT[n]], lhsT,
                            w2s[:, 2 * kk:2 * kk + 2, NOFF[n]:NOFF[n] + NT[n]],
                            start=(kk == 0),
                            stop=(kk == KT // 2 - 1 and n < NDVE),
                            perf_mode=dr,
                        )
                        if n > 0:
                            # consecutive matmuls reuse the array-resident
                            # stationary (same lhsT) -- skip the reload
                            mm.ins.ldweights = False
                # rank-1 +b2 for the ACT-evicted tiles (ones x 2^11*b2)
                for n in range(NDVE, 8):
                    mm = nc.tensor.matmul(
                        pts[n][:, :NT[n]], ones1[:],
                        b2ms[:, NOFF[n]:NOFF[n] + NT[n]],
                        start=False, stop=True,
                    )
                    if n > NDVE:
                        mm.ins.ldweights = False
                for n in range(NDVE):
                    nc.vector.scalar_tensor_tensor(
                        o[:, NOFF[n]:NOFF[n] + NT[n]], pts[n][:, :NT[n]], OSC,
                        b2s[:, NOFF[n]:NOFF[n] + NT[n]], op0=mul_op, op1=add_op)
                for n in range(NDVE, 8):
                    nc.scalar.activation(
                        o[:, NOFF[n]:NOFF[n] + NT[n]], pts[n][:, :NT[n]],
                        copyf, scale=OSC)
            r0 = c * CHUNK
            dst = out.ap()[r0:r0 + CHUNK, :].rearrange(
                "(mb p) v -> p mb v", mb=CHUNK // P, p=P)
            nc.sync.dma_start(dst, ob[:])

    nc.compile()
    return nc


def get_nc():
    global _CACHED_NC
    if _CACHED_NC is None:
        _CACHED_NC = _build_nc()
    return _CACHED_NC


def _prep_inputs(tokens_seq, W1t, b1, W2, b2):
    """Host-side sharding/layout. Returns in_maps for the 8 cores."""
    tokens = np.asarray(tokens_seq).astype(np.int64)
    assert tokens.shape == (SEQ, BATCH)
    W1t = np.asarray(W1t, dtype=np.float32)
    b1 = np.asarray(b1, dtype=np.float32)
    W2 = np.asarray(W2, dtype=np.float32)
    b2 = np.asarray(b2, dtype=np.float32)

    padded = np.concatenate(
        [np.zeros((KGRAM - 1, BATCH), dtype=np.int64), tokens], axis=0)
    uniq, inv = np.unique(padded, return_inverse=True)
    inv = inv.reshape(padded.shape)
    U = len(uniq)
    assert U <= UPAD

    # compact bf16 table: row j*UPAD + u  <-  W1t[j*VOCAB + uniq[u]] + b1/3
    # (the b1/3 bias makes the 3-row sum equal emb + b1 directly)
    b1third = (b1 / np.float32(KGRAM))[None, :]
    table = np.zeros((RTAB, EMBED), dtype=_BF16)
    for j in range(KGRAM):
        table[j * UPAD:j * UPAD + U] = (W1t[j * VOCAB + uniq] + b1third).astype(_BF16)

    # gather index stream per chunk: [j=0 positions..., j=1 ..., j=2 ...]
    # cid(j, pos) = j*UPAD + inv[s+j, b] with pos = s*BATCH + b
    cidx = np.empty((KGRAM, POS), dtype=np.int16)
    for j in range(KGRAM):
        cidx[j] = (j * UPAD + inv[j:j + SEQ, :]).reshape(-1).astype(np.int16)
    idx_host = np.empty((P, NCH * IDXC), dtype=np.int16)
    for c in range(NCH):
        stream = np.concatenate(
            [cidx[j, c * CHUNK:(c + 1) * CHUNK] for j in range(KGRAM)])
        blk = stream.reshape(IDXC, 16).T      # [i%16, i//16]
        idx_host[:, c * IDXC:(c + 1) * IDXC] = np.tile(blk, (8, 1))

    w2r = (np.float32(WSC) * W2).reshape(KT, P, VOCAB)
    in_maps = []
    for core in range(NCORES):
        v0 = core * VSH
        w2b = np.ascontiguousarray(
            w2r[:, :, v0:v0 + VSH].transpose(1, 0, 2)).reshape(P, KT * VSH)
        in_maps.append({
            "table": table,
            "idx": idx_host,
            "w2b": w2b.astype(_FP8),
            "b2r": np.ascontiguousarray(
                np.broadcast_to(b2[v0:v0 + NOFF[NDVE]], (P, NOFF[NDVE]))),
            "b2m": (np.float32(HSC * WSC) * b2[v0:v0 + VSH])
                   .astype(_BF16).reshape(1, VSH),
        })
    return in_maps


def run(tokens_seq, W1t, b1, W2, b2, trace=False):
    """Run on 8 cores; returns (logits, BassKernelResults)."""
    from concourse.bass_utils import run_bass_kernel_spmd

    nc = get_nc()
    in_maps = _prep_inputs(tokens_seq, W1t, b1, W2, b2)
    res = run_bass_kernel_spmd(nc, in_maps, list(range(NCORES)), trace=trace)
    parts = [res.results[i]["out"] for i in range(NCORES)]
    logits = np.concatenate(parts, axis=1).astype(np.float32)
    return logits.reshape(SEQ, BATCH, VOCAB), res


def kernel(tokens_seq, W1t, b1, W2, b2):
    logits, _ = run(tokens_seq, W1t, b1, W2, b2)
    return logits
